# revision 1
# baseline (speedup 1.0000x reference)
"""Trainium2 Bass kernel for nn_DeepSSM: LSTM over [B=256, T=2048, obs=32] -> [B, T, 64].

Strategy
--------
Data-parallel: batch 256 -> 8 cores x 32. Per core, the 32-batch is split into
G=2 independent 16-batch chains that are software-pipelined to hide the
per-step dependency latency of the recurrence.

Everything on-chip runs in a "transposed" layout: gates live in PSUM as
[gate_idx (partitions), batch (free)], hidden/cell state as [hid, batch].
Gate columns are permuted into two 128-wide chunks: chunk1 = [i; g],
chunk2 = [f; o], and the i/f/o weight columns are pre-scaled by 0.5 so that a
single Tanh activation serves all four gates (sigmoid(x) = (1+tanh(x/2))/2).

Per 16-step window and chain, one PSUM bank holds the gate pre-activations:
cols 0:256 = chunk1 (tau-major), cols 256:512 = chunk2. Two x-projection
matmuls fill it (start=True on the first; the second accumulates onto the
bank's pending-zero region; an explicit no-sync dep keeps their order), then
per-step recurrent matmuls accumulate Wh*h. The bias rides a ones-row of x.

x is staged in a never-reused persistent SBUF region (64KB/partition per
chain) so the per-window x DMAs carry no data waits: the restrictive
DIRECT2D DMA fast path allows only the queue semaphore.

Per chain and timestep (stock ops only - custom DVE ops don't compile with
this walrus, and two-SBUF-input DVE ops must share a base partition):
  PE   : 2 matmuls (Wh_cA/Wh_cB @ h') accumulating onto the x-projection.
  ACT  : 1 tanh over both gate chunks (interleaved output); 1 tanh(0.5*y)
         for the cell state (y = 2c tracked to fold the sigmoid halves).
  DVE  : rebase copy of the o/g half to partition 0; paired mult+add
         -> S = [(1+t_f)y | (1+t_i)t_g] interleaved; pairwise
         tensor_tensor_scan (d0 = [0, .5]) -> y' = S_i + S_f/2; then
         h' = 2h = (1+t_o)tanh(c') via mult+add (Wh pre-halved on host,
         output halved on host).

Host side pre-transposes x and post-transposes the output, so the device
never transposes anything.
"""

import os
import numpy as np
import ml_dtypes

BF16 = ml_dtypes.bfloat16

OBS = 32
HID = 64
T_FULL = 2048
B_FULL = 256
N_CORES = 8
BPC = B_FULL // N_CORES  # 32 batch per core
G = int(os.environ.get("LSTM_G", "2"))   # chains per core
BG = BPC // G            # batch per chain
WIN = 512 // (2 * BG)    # timesteps per PSUM window (WIN * 2 * BG = 512 cols)
KA = OBS + 1             # x rows incl ones-row

_NC_CACHE = {}


# --------------------------------------------------------------------------
# Custom DVE ops
# --------------------------------------------------------------------------
_OPS_REGISTERED = False
PAIRPROD = None  # out = s0 * (1 + in0) * in1
TANHPOLY = None  # out = clamp(x*(s0 + s1*x^2 + imm2*x^4), -1, 1)  ~ tanh(x)
# Minimax fit of tanh via output-clamped odd quintic (max abs err ~1.9e-2).
TANH_C = (0.9312120465782658, -0.1763841940228923, 0.015448984744725808)


def _register_dve_ops():
    global _OPS_REGISTERED, PAIRPROD, TANHPOLY
    if _OPS_REGISTERED:
        return
    import concourse.dve_ops as dve_ops
    from concourse.dve_ops import DveOp
    from concourse.dve_spec import (Spec, Src0, Src1, C0, C1, C2, One, Zero,
                                    minn, maxx, sq, lower, _has_src1)
    from concourse.dve_uop import DveOpSpec

    def _make(name, spec):
        existing = next((op for op in dve_ops.OPS if op.name == name), None)
        if existing is not None:
            return existing
        row = dve_ops._CUSTOM_DVE_ROW_BASE + len(dve_ops.OPS)
        dve_ops._SUB_OPCODE_FOR_NAME[name] = row
        shas = {}
        for ver in ("v3", "v4"):
            s = DveOpSpec(name=name, opcode=row, uops=lower(spec, ver=ver),
                          rd1_en=_has_src1(spec))
            shas[ver] = s.sha(ver)
        op = DveOp(name, spec, subdim=False, uops_sha=shas)
        dve_ops.OPS.append(op)
        dve_ops.CUSTOM_DVE_SPECS[name] = spec
        return op

    PAIRPROD = _make("LSTM_PAIRPROD_ANT", Spec(
        body=(Src0 + One) * Src1 * C0,
        reference=lambda in0, in1, s0, s1, imm2: (
            (in0.astype(np.float32) + 1.0)
            * np.asarray(in1, np.float32).reshape(in0.shape) * s0
        ),
    ))

    z = sq(Src0)
    p = Src0 * (C0 + z * (C1 + z * C2))
    TANHPOLY = _make("LSTM_TANHPOLY_ANT", Spec(
        body=maxx(minn(p, One), Zero - One),
        reference=lambda in0, in1, s0, s1, imm2: np.clip(
            in0.astype(np.float32)
            * (s0 + in0.astype(np.float32) ** 2
               * (s1 + in0.astype(np.float32) ** 2 * imm2)), -1.0, 1.0),
    ))
    _OPS_REGISTERED = True


# --------------------------------------------------------------------------
# Device program
# --------------------------------------------------------------------------
def build_nc(t_steps=T_FULL, n_dve_tanh=int(os.environ.get("LSTM_DVE_TANH", "0"))):
    """Build the Bass program for one core (all cores run the same NEFF).

    n_dve_tanh: number of chains (0..G) whose cell-state tanh runs as a
    polynomial approximation on the Vector engine instead of ScalarE.
    """
    _register_dve_ops()
    import concourse.bass as bass
    import concourse.tile as tile
    import concourse.mybir as mybir
    from concourse.tile import add_dep_helper

    f32 = mybir.dt.float32
    bf16 = mybir.dt.bfloat16
    TANH = mybir.ActivationFunctionType.Tanh

    n_win = t_steps // WIN
    SW = 2 * BG              # bank columns per step across both chunks
    NW = WIN * BG            # bank columns per chunk per window (256)
    nc = bass.Bass("TRN2", debug=False, num_devices=N_CORES,
                   enable_partition_id=False)

    # DRAM I/O (per core). x: [KA, T, BG] per chain ([x; ones] rows).
    x_dram = [nc.dram_tensor(f"x{g}", [KA, t_steps, BG], bf16,
                             kind="ExternalInput") for g in range(G)]
    # All weights in one tensor/DMA: cols 0:128 = wx_c1, 128:256 = wx_c2
    # (rows 0:KA), 256:384 = wh_c1, 384:512 = wh_c2 (rows 0:64).
    wcat = nc.dram_tensor("wcat", [HID, 512], bf16, kind="ExternalInput")
    out_dram = [nc.dram_tensor(f"h{g}", [HID, t_steps, BG], bf16,
                               kind="ExternalOutput") for g in range(G)]

    with tile.TileContext(nc) as tc:
        from contextlib import ExitStack
        ctx = ExitStack()
        with ctx:
            wpool = ctx.enter_context(tc.tile_pool(name="weights", bufs=1))
            tpool = [ctx.enter_context(tc.tile_pool(name=f"T{g}", bufs=6))
                     for g in range(G)]
            wprod = [ctx.enter_context(tc.tile_pool(name=f"W{g}", bufs=4))
                     for g in range(G)]
            tcpool = [ctx.enter_context(tc.tile_pool(name=f"tc{g}", bufs=4))
                      for g in range(G)]
            hpool = [ctx.enter_context(tc.tile_pool(name=f"h{g}", bufs=3))
                     for g in range(G)]
            bankp = [ctx.enter_context(
                tc.tile_pool(name=f"psum{g}", bufs=2, space="PSUM"))
                for g in range(G)]

            w_all = wpool.tile([HID, 512], bf16)
            nc.sync.dma_start(w_all[:, :], wcat[:, :])
            wx1_ap = w_all[0:KA, 0:128]
            wx2_ap = w_all[0:KA, 128:256]
            wh1_ap = w_all[:, 256:384]
            wh2_ap = w_all[:, 384:512]
            # PE observes the weights DMA once so no later matmul needs a
            # sync-wait slot for it.
            nc.tensor.ldweights(wh1_ap)

            # Never-reused x staging region: per-window DMAs into distinct
            # slices carry no data waits (DIRECT2D DMAs only get one).
            xreg = [nc.alloc_sbuf_tensor(f"xreg{g}", [KA, t_steps * BG], bf16)
                    for g in range(G)]

            # Scan multiplier pattern [0, 0.5, 0, 0.5, ...]: resets the scan
            # state at each pair's first element, halves it at the second.
            scanc_d = nc.dram_tensor("scanc", [HID, SW], f32,
                                     kind="ExternalInput")
            scanc = wpool.tile([HID, SW], f32)
            nc.sync.dma_start(scanc[:, :], scanc_d[:, :])

            EXT = 2 * BG      # T-tile ext region width (scan out, y at odds)
            h_prev = []
            T_cur = []
            banks = [[None, None] for _ in range(G)]
            h_win = [None] * G

            for g in range(G):
                h0 = hpool[g].tile([HID, BG], bf16, tag="hinit")
                nc.vector.memset(h0[:, :], 0.0)
                h_prev.append(h0[:, :])
                t0 = tpool[g].tile([128, 3 * EXT], f32)
                nc.vector.memset(t0[0:64, 0:EXT], 0.0)  # y_0 = 2*c_0 = 0
                T_cur.append(t0)

            def start_window(g, w):
                """One DMA + two ordered matmuls: project x into a bank."""
                xw = xreg[g][:][:, w * NW:(w + 1) * NW]
                src = x_dram[g][:, w * WIN:(w + 1) * WIN, :]
                nc.sync.dma_start(xw, src.rearrange("p t b -> p (t b)"))
                bank = bankp[g].tile([128, 2 * NW], f32)
                mm1 = nc.tensor.matmul(bank[:, 0:NW], lhsT=wx1_ap, rhs=xw,
                                       start=True, stop=False,
                                       skip_group_check=True)
                mm2 = nc.tensor.matmul(bank[:, NW:2 * NW], lhsT=wx2_ap,
                                       rhs=xw, start=False, stop=False,
                                       skip_group_check=True)
                # Keep the bank-clearing mm first; same engine, no sem.
                add_dep_helper(mm2.ins, mm1.ins, sync=False,
                               reason="xproj order after bank clear")
                banks[g][w % 2] = bank

            for g in range(G):
                start_window(g, 0)

            for w in range(n_win):
                for g in range(G):
                    if w + 1 < n_win:
                        start_window(g, w + 1)
                    h_win[g] = hpool[g].tile([HID, WIN * BG], bf16,
                                             name=f"hwin{g}_{w}", tag="hwin")
                for tau in range(WIN):
                    for g in range(G):
                        bank = banks[g][w % 2]
                        cA = bank[:, tau * BG:(tau + 1) * BG]
                        cB = bank[:, NW + tau * BG:NW + (tau + 1) * BG]
                        last = tau == WIN - 1
                        nc.tensor.matmul(cA, lhsT=wh1_ap, rhs=h_prev[g],
                                         start=False, stop=False,
                                         skip_group_check=True)
                        nc.tensor.matmul(cB, lhsT=wh2_ap, rhs=h_prev[g],
                                         start=False, stop=last,
                                         skip_group_check=True)
                        Tc = T_cur[g]
                        # T layout (all pair math at base partition 0):
                        # cols 0:EXT        p<64: ext (y=2c' at odd slots)
                        # cols EXT:2EXT     p<64: copy of o@even/g@odd half
                        # cols 2EXT:3EXT    tanh(gates) interleaved
                        #   (p<64: f@even, i@odd; p>=64: o@even, g@odd)
                        act_in = bank[:, :].rearrange(
                            "p (c n) -> p c n", c=2)[:, :,
                                                     tau * BG:(tau + 1) * BG]
                        act_out = Tc[:, 2 * EXT:3 * EXT].rearrange(
                            "p (n c) -> p c n", c=2)
                        nc.scalar.activation(act_out, act_in, TANH)
                        # rebase the o/g half to partition 0 (walrus forbids
                        # two-SBUF-input ops with differing base partitions)
                        nc.vector.tensor_copy(Tc[0:64, EXT:2 * EXT],
                                              Tc[64:128, 2 * EXT:3 * EXT])

                        Tn = tpool[g].tile([128, 3 * EXT], f32)
                        Mt = wprod[g].tile([HID, SW], f32, tag="m")
                        St = wprod[g].tile([HID, SW], f32, tag="s")
                        # pairs: f<->y (=2c), i<->g
                        src0 = Tc[0:64, 2 * EXT:3 * EXT].rearrange(
                            "p (n c) -> p c n", c=2)          # f's then i's
                        src1 = Tc[0:64, 0:2 * EXT].rearrange(
                            "p (b n c) -> p b c n", b=2, c=2)[:, :, 1, :]
                        nc.vector.tensor_tensor(Mt[:, :], src0, src1,
                                                mybir.AluOpType.mult)
                        nc.vector.tensor_tensor(
                            St[:, :].rearrange("p (n c) -> p c n", c=2),
                            Mt[:, :], src1, mybir.AluOpType.add)
                        # y' = S_i + 0.5*S_f via pairwise scan (d0=[0,.5])
                        nc.vector.tensor_tensor_scan(
                            Tn[0:64, 0:EXT], scanc[:, :], St[:, :], 0.0,
                            mybir.AluOpType.mult, mybir.AluOpType.add)
                        tct = tcpool[g].tile([HID, BG], f32)
                        nc.scalar.activation(
                            tct[:, :],
                            Tn[0:64, 0:EXT].rearrange(
                                "p (n c) -> p c n", c=2)[:, 1, :],
                            TANH, scale=0.5)
                        # h' = 2h = (1+t_o)*tanh(c'); Wh is pre-halved and
                        # the host halves the output.
                        h_sl = h_win[g][:, tau * BG:(tau + 1) * BG]
                        t_o = Tc[0:64, EXT:2 * EXT].rearrange(
                            "p (n c) -> p c n", c=2)[:, 0, :]
                        m2 = tcpool[g].tile([HID, BG], f32, tag="m2")
                        nc.vector.tensor_tensor(m2[:, :], t_o, tct[:, :],
                                                mybir.AluOpType.mult)
                        nc.vector.tensor_tensor(h_sl, m2[:, :], tct[:, :],
                                                mybir.AluOpType.add)
                        h_prev[g] = h_sl
                        T_cur[g] = Tn
                for g in range(G):
                    dst = out_dram[g][:, w * WIN:(w + 1) * WIN, :]
                    nc.sync.dma_start(dst.rearrange("p t b -> p (t b)"),
                                      h_win[g][:, :])
    return nc


def _split_waits(nc, mybir, nmax=1):
    """This walrus accepts only one sync-wait per instruction: move excess
    waits onto preceding same-engine NOPs."""
    fn = nc.m.functions[0]
    for bb in fn.blocks:
        newlist = []
        for ins in bb.instructions:
            si = getattr(ins, "sync_info", None)
            if si is not None and si.on_wait and len(si.on_wait) > nmax:
                waits = list(si.on_wait)
                while len(waits) > nmax:
                    chunk, waits = waits[:nmax], waits[nmax:]
                    nop = mybir.InstNoOp(
                        name=nc.get_next_instruction_name(), ins=[], outs=[])
                    nop.engine = ins.engine
                    nop.sync_info = mybir.SyncInfo(on_wait=chunk, on_update=[])
                    newlist.append(nop)
                si.on_wait = waits
            newlist.append(ins)
        bb.instructions[:] = newlist


# --------------------------------------------------------------------------
# Host-side weight/input prep
# --------------------------------------------------------------------------
def _prep_weights(Wx, Wh, b):
    """Permute gate columns into chunks [i;g] and [f;o]; scale i/f/o by 0.5;
    fold the bias into an extra row of Wx; stack everything into wcat."""
    H = HID
    idx_i = np.arange(0, H)
    idx_f = np.arange(H, 2 * H)
    idx_g = np.arange(2 * H, 3 * H)
    idx_o = np.arange(3 * H, 4 * H)
    scale = np.ones(4 * H, np.float32)
    scale[np.concatenate([idx_i, idx_f, idx_o])] = 0.5
    Wxs = (np.asarray(Wx, np.float32) * scale)
    Whs = (np.asarray(Wh, np.float32) * scale)
    bs = (np.asarray(b, np.float32) * scale)
    Wxa = np.concatenate([Wxs, bs[None, :]], axis=0)  # [KA, 256]
    c1 = np.concatenate([idx_i, idx_g])
    c2 = np.concatenate([idx_f, idx_o])
    wcat = np.zeros((HID, 512), np.float32)
    wcat[0:KA, 0:128] = Wxa[:, c2]      # chunk A = [f; o]
    wcat[0:KA, 128:256] = Wxa[:, c1]    # chunk B = [i; g]
    # Recurrent weights additionally halved: the device recurrence carries
    # h' = 2h (the host halves the output), so Wh_dev = Wh_scaled / 2.
    wcat[:, 256:384] = Whs[:, c2] * 0.5
    wcat[:, 384:512] = Whs[:, c1] * 0.5
    return wcat.astype(BF16)


def _prep_x(y_core):
    """y_core [BPC, T, OBS] fp32 -> per chain [KA, T, BG] bf16 ([x; 1])."""
    t_steps = y_core.shape[1]
    xt = y_core.transpose(2, 1, 0)  # [OBS, T, BPC]
    out = []
    for g in range(G):
        xa = np.empty((KA, t_steps, BG), np.float32)
        xa[0:OBS] = xt[:, :, g * BG:(g + 1) * BG]
        xa[OBS] = 1.0
        out.append(np.ascontiguousarray(xa.astype(BF16)))
    return out


def kernel(y, Wx, Wh, b):
    from concourse.bass_utils import run_bass_kernel_spmd

    y = np.asarray(y)
    t_steps = y.shape[1]
    wcat = _prep_weights(Wx, Wh, b)

    key = t_steps
    if key not in _NC_CACHE:
        import concourse.mybir as mybir
        nc = build_nc(t_steps)
        _split_waits(nc, mybir)   # CoreSim can't run the split form
        _NC_CACHE[key] = nc
    nc = _NC_CACHE[key]

    scanc = np.zeros((HID, 2 * BG), np.float32)
    scanc[:, 1::2] = 0.5
    in_maps = []
    for c in range(N_CORES):
        xs = _prep_x(y[c * BPC:(c + 1) * BPC])
        m = {"wcat": wcat, "scanc": scanc}
        for g in range(G):
            m[f"x{g}"] = xs[g]
        in_maps.append(m)

    res = run_bass_kernel_spmd(
        nc, in_maps, core_ids=list(range(N_CORES)),
        trace=bool(int(os.environ.get("LSTM_TRACE", "0"))))

    out = np.empty((B_FULL, t_steps, HID), np.float32)
    for c in range(N_CORES):
        for g in range(G):
            hg = res.results[c][f"h{g}"].astype(np.float32)  # [HID, T, BG]
            out[c * BPC + g * BG:c * BPC + (g + 1) * BG] = (
                hg.transpose(2, 1, 0) * 0.5)
    globals()["_LAST_RESULT"] = res
    return out



# revision 5
# speedup vs baseline: 1.3679x; 1.3679x over previous
"""Trainium2 Bass kernel for nn_DeepSSM: LSTM over [B=256, T=2048, obs=32] -> [B, T, 64].

Strategy
--------
Data-parallel: batch 256 -> 8 cores x 32. Per core, the 32-batch is split into
G=2 independent 16-batch chains, software-pipelined to hide the per-step
dependency latency of the recurrence.

On-chip layout is "transposed": gates in PSUM as [gate_unit (partitions 0:64),
cols], hidden/cell state as [hid, batch]. The four gates are kept as four
SEPARATE 64-column weight chunks; the four per-step recurrent matmuls write
each gate to partitions 0:64 of the bank (gate-blocked columns
col = 64*tau + 16*c + n), so every downstream elementwise op runs at base
partition 0 and the old cross-partition rebase copy disappears.

i/f/o weight columns are pre-scaled by 0.5 so one Tanh activation serves all
four gates (sigmoid(x) = (1+tanh(x/2))/2); the recurrence tracks y = 2c and
h' = 2h (Wh pre-halved on host, output halved on host).

Per step and chain (t-th step uses T-buffer t%2, writes y' into (t+1)%2):
  PE   : 4 matmuls (Wh_f/i/g/o @ h') accumulating onto the x-projection.
  ACT  : tanh over the 4 gate chunks -> T cols 5n+{1..4} = (t_f,t_i,t_g,t_o);
         tanh(0.5*y') -> tct.
  DVE  : 3 fused scalar_tensor_tensor ops:
           V    = ((t_f,t_i) + 1) * (y, t_g)     [pairs, one op]
           y'   = 0.5*V_f + V_i                  -> T_next col 5n+0
           h'   = (t_o + 1) * tct                -> h_win (bf16)

T layout per chain: two static [64, 5*BG] f32 buffers, cols 5n+0 = y (prev
step's 2c), 5n+1..4 = tanh(f,i,g,o). Static buffers (not pool tiles) keep
the access patterns' relative offsets constant; WAR reuse at distance 2 steps
is transitively ordered through the recurrence itself.

x is staged in a never-reused persistent SBUF region so the per-window x DMAs
carry no data waits. Host pre-transposes x and post-transposes the output.
"""

import os
import numpy as np
import ml_dtypes

BF16 = ml_dtypes.bfloat16

OBS = 32
HID = 64
T_FULL = 2048
B_FULL = 256
N_CORES = 8
BPC = B_FULL // N_CORES  # 32 batch per core
G = int(os.environ.get("LSTM_G", "2"))   # chains per core
BG = BPC // G            # batch per chain
WIN = 512 // (4 * BG)    # timesteps per PSUM window (WIN * 4 * BG = 512 cols)
KA = OBS + 1             # x rows incl ones-row

_NC_CACHE = {}


# --------------------------------------------------------------------------
# Device program
# --------------------------------------------------------------------------
def build_nc(t_steps=T_FULL):
    """Build the Bass program for one core (all cores run the same NEFF)."""
    import concourse.bass as bass
    import concourse.tile as tile
    import concourse.mybir as mybir
    from concourse.tile import add_dep_helper

    f32 = mybir.dt.float32
    bf16 = mybir.dt.bfloat16
    TANH = mybir.ActivationFunctionType.Tanh
    ADD = mybir.AluOpType.add
    MULT = mybir.AluOpType.mult

    n_win = t_steps // WIN
    NWC = WIN * 4 * BG       # bank columns per window (512)
    nc = bass.Bass("TRN2", debug=False, num_devices=N_CORES,
                   enable_partition_id=False)

    # DRAM I/O (per core). x: [KA, T, BG] per chain ([x; ones] rows).
    x_dram = [nc.dram_tensor(f"x{g}", [KA, t_steps, BG], bf16,
                             kind="ExternalInput") for g in range(G)]
    # Weights: cols 0:256 = Wx chunks (rows 0:KA) in gate order (f,i,g,o),
    # cols 256:512 = Wh chunks (rows 0:64), pre-halved.
    wcat = nc.dram_tensor("wcat", [HID, 512], bf16, kind="ExternalInput")
    out_dram = [nc.dram_tensor(f"h{g}", [HID, t_steps, BG], bf16,
                               kind="ExternalOutput") for g in range(G)]

    with tile.TileContext(nc) as tc:
        from contextlib import ExitStack
        ctx = ExitStack()
        with ctx:
            wpool = ctx.enter_context(tc.tile_pool(name="weights", bufs=1))
            hpool = [ctx.enter_context(tc.tile_pool(name=f"h{g}", bufs=3))
                     for g in range(G)]
            bankp = [ctx.enter_context(
                tc.tile_pool(name=f"psum{g}", bufs=2, space="PSUM"))
                for g in range(G)]

            w_all = wpool.tile([HID, 512], bf16)
            nc.sync.dma_start(w_all[:, :], wcat[:, :])
            wx_c = [w_all[0:KA, 64 * c:64 * (c + 1)] for c in range(4)]
            wh_c = [w_all[:, 256 + 64 * c:256 + 64 * (c + 1)]
                    for c in range(4)]
            # PE observes the weights DMA once so no later matmul needs a
            # sync-wait slot for it.
            nc.tensor.ldweights(wh_c[0])

            # Never-reused x staging region: per-window DMAs into distinct
            # slices carry no data waits.
            xreg = [nc.alloc_sbuf_tensor(f"xreg{g}", [KA, t_steps * BG], bf16)
                    for g in range(G)]

            # Static per-chain state: double-buffered T (y + gate tanh),
            # pair-product scratch W, cell tanh tct.
            Tst = [[nc.alloc_sbuf_tensor(f"T{g}_{b}", [HID, 5 * BG], f32)
                    for b in range(2)] for g in range(G)]
            Wst = [nc.alloc_sbuf_tensor(f"W{g}", [HID, 2 * BG], f32)
                   for g in range(G)]
            tct = [nc.alloc_sbuf_tensor(f"tct{g}", [HID, BG], f32)
                   for g in range(G)]

            h_prev = []
            banks = [[None, None] for _ in range(G)]
            h_win = [None] * G

            for g in range(G):
                h0 = hpool[g].tile([HID, BG], bf16, tag="hinit")
                nc.vector.memset(h0[:, :], 0.0)
                h_prev.append(h0[:, :])
                for b in range(2):
                    nc.vector.memset(Tst[g][b][:], 0.0)  # y_0 = 2*c_0 = 0

            def start_window(g, w):
                """One DMA + four ordered matmuls: project x into a bank."""
                xw = xreg[g][:][:, w * WIN * BG:(w + 1) * WIN * BG]
                src = x_dram[g][:, w * WIN:(w + 1) * WIN, :]
                nc.sync.dma_start(xw, src.rearrange("p t b -> p (t b)"))
                bank = bankp[g].tile([HID, NWC], f32)
                # col = 128*c + 16*tau + n; chunk c owns a contiguous block
                mm0 = None
                for c in range(4):
                    blk = bank[:, c * WIN * BG:(c + 1) * WIN * BG]
                    mm = nc.tensor.matmul(blk, lhsT=wx_c[c],
                                          rhs=xw, start=(c == 0), stop=False,
                                          skip_group_check=True)
                    if c == 0:
                        mm0 = mm
                    else:
                        # Keep the bank-clearing mm first; same engine, no
                        # sem needed.
                        add_dep_helper(mm.ins, mm0.ins, sync=False,
                                       reason="xproj order after bank clear")
                banks[g][w % 2] = bank
                return bank

            for g in range(G):
                start_window(g, 0)

            for w in range(n_win):
                for g in range(G):
                    if w + 1 < n_win:
                        start_window(g, w + 1)
                    h_win[g] = hpool[g].tile([HID, WIN * BG], bf16,
                                             name=f"hwin{g}_{w}", tag="hwin")
                for tau in range(WIN):
                    t = w * WIN + tau
                    for g in range(G):
                        bank = banks[g][w % 2]
                        last = tau == WIN - 1
                        for c in range(4):
                            blk = bank[:, c * WIN * BG + tau * BG:
                                       c * WIN * BG + (tau + 1) * BG]
                            nc.tensor.matmul(
                                blk, lhsT=wh_c[c],
                                rhs=h_prev[g], start=False,
                                stop=(last and c == 3),
                                skip_group_check=True)
                        Tb = Tst[g][t % 2][:].rearrange(
                            "p (n f) -> p n f", f=5)
                        Tn = Tst[g][(t + 1) % 2][:].rearrange(
                            "p (n f) -> p n f", f=5)
                        W3 = Wst[g][:].rearrange("p (n c) -> p n c", c=2)
                        # gate tanh: bank (n,c) iteration -> T cols 5n+1+c
                        bank_nc = bank[:, :].rearrange(
                            "p (c t n) -> p t n c", c=4, n=BG)[:, tau]
                        nc.scalar.activation(Tb[:, :, 1:5], bank_nc, TANH)
                        # V = ((t_f,t_i)+1) * (y, t_g)
                        nc.vector.scalar_tensor_tensor(
                            W3[:, :, :], Tb[:, :, 1:3], 1.0,
                            Tb[:, :, 0::3], ADD, MULT)
                        # y' = 0.5*V_f + V_i -> T_next col 5n+0
                        nc.vector.scalar_tensor_tensor(
                            Tn[:, :, 0], W3[:, :, 0], 0.5,
                            W3[:, :, 1], MULT, ADD)
                        # tct = tanh(0.5*y')
                        nc.scalar.activation(tct[g][:], Tn[:, :, 0],
                                             TANH, scale=0.5)
                        # h' = (t_o+1) * tct  (bf16)
                        h_sl = h_win[g][:, tau * BG:(tau + 1) * BG]
                        nc.vector.scalar_tensor_tensor(
                            h_sl, Tb[:, :, 4], 1.0, tct[g][:], ADD, MULT)
                        h_prev[g] = h_sl
                for g in range(G):
                    dst = out_dram[g][:, w * WIN:(w + 1) * WIN, :]
                    nc.sync.dma_start(dst.rearrange("p t b -> p (t b)"),
                                      h_win[g][:, :])
    return nc


def _split_waits(nc, mybir, nmax=1):
    """This walrus accepts only one sync-wait per instruction: move excess
    waits onto preceding same-engine NOPs."""
    fn = nc.m.functions[0]
    for bb in fn.blocks:
        newlist = []
        for ins in bb.instructions:
            si = getattr(ins, "sync_info", None)
            if si is not None and si.on_wait and len(si.on_wait) > nmax:
                waits = list(si.on_wait)
                while len(waits) > nmax:
                    chunk, waits = waits[:nmax], waits[nmax:]
                    nop = mybir.InstNoOp(
                        name=nc.get_next_instruction_name(), ins=[], outs=[])
                    nop.engine = ins.engine
                    nop.sync_info = mybir.SyncInfo(on_wait=chunk, on_update=[])
                    newlist.append(nop)
                si.on_wait = waits
            newlist.append(ins)
        bb.instructions[:] = newlist


# --------------------------------------------------------------------------
# Host-side weight/input prep
# --------------------------------------------------------------------------
def _prep_weights(Wx, Wh, b):
    """Scale i/f/o columns by 0.5, fold the bias into an extra row of Wx,
    split into per-gate 64-col chunks in device order (f,i,g,o), halve Wh."""
    H = HID
    idx = {"i": np.arange(0, H), "f": np.arange(H, 2 * H),
           "g": np.arange(2 * H, 3 * H), "o": np.arange(3 * H, 4 * H)}
    scale = np.ones(4 * H, np.float32)
    scale[np.concatenate([idx["i"], idx["f"], idx["o"]])] = 0.5
    Wxs = np.asarray(Wx, np.float32) * scale
    Whs = np.asarray(Wh, np.float32) * scale
    bs = np.asarray(b, np.float32) * scale
    Wxa = np.concatenate([Wxs, bs[None, :]], axis=0)  # [KA, 256]
    order = ["f", "i", "g", "o"]
    wcat = np.zeros((HID, 512), np.float32)
    for c, gate in enumerate(order):
        wcat[0:KA, 64 * c:64 * (c + 1)] = Wxa[:, idx[gate]]
        # Recurrent weights additionally halved: the device recurrence
        # carries h' = 2h (the host halves the output).
        wcat[:, 256 + 64 * c:256 + 64 * (c + 1)] = Whs[:, idx[gate]] * 0.5
    return wcat.astype(BF16)


def _prep_x(y_core):
    """y_core [BPC, T, OBS] fp32 -> per chain [KA, T, BG] bf16 ([x; 1])."""
    t_steps = y_core.shape[1]
    xt = y_core.transpose(2, 1, 0)  # [OBS, T, BPC]
    out = []
    for g in range(G):
        xa = np.empty((KA, t_steps, BG), np.float32)
        xa[0:OBS] = xt[:, :, g * BG:(g + 1) * BG]
        xa[OBS] = 1.0
        out.append(np.ascontiguousarray(xa.astype(BF16)))
    return out


def kernel(y, Wx, Wh, b):
    from concourse.bass_utils import run_bass_kernel_spmd

    y = np.asarray(y)
    t_steps = y.shape[1]
    wcat = _prep_weights(Wx, Wh, b)

    key = t_steps
    if key not in _NC_CACHE:
        import concourse.mybir as mybir
        nc = build_nc(t_steps)
        _split_waits(nc, mybir)   # CoreSim can't run the split form
        _NC_CACHE[key] = nc
    nc = _NC_CACHE[key]

    in_maps = []
    for c in range(N_CORES):
        xs = _prep_x(y[c * BPC:(c + 1) * BPC])
        m = {"wcat": wcat}
        for g in range(G):
            m[f"x{g}"] = xs[g]
        in_maps.append(m)

    res = run_bass_kernel_spmd(
        nc, in_maps, core_ids=list(range(N_CORES)),
        trace=bool(int(os.environ.get("LSTM_TRACE", "0"))))

    out = np.empty((B_FULL, t_steps, HID), np.float32)
    for c in range(N_CORES):
        for g in range(G):
            hg = res.results[c][f"h{g}"].astype(np.float32)  # [HID, T, BG]
            out[c * BPC + g * BG:c * BPC + (g + 1) * BG] = (
                hg.transpose(2, 1, 0) * 0.5)
    globals()["_LAST_RESULT"] = res
    return out


# revision 18
# speedup vs baseline: 3.8462x; 2.8117x over previous
"""Trainium2 Bass kernel for nn_DeepSSM: LSTM over [B=256, T=2048, obs=32] -> [B, T, 64].

Strategy: Picard iteration (batch-parallel-in-time)
---------------------------------------------------
Data-parallel: batch 256 -> 8 cores x 32 lanes. A sequential LSTM on this
hardware is latency-wall bound (~1.7us per step of engine round-trips x 2048
steps). Instead, iterate the fixed-point map

    h^{m}(t) = LSTMStep(x(t), h^{m-1}(t-1))          (all t in parallel)

which contracts at ~0.25x per sweep (the h-feedback through Wh is a weak
coupling; the c-recurrence given the gates is a first-order linear scan that
tensor_tensor_scan computes exactly, fp32 state). 5 sweeps reach ~4e-3
relative error - the same territory as the bf16 sequential kernel.

All-sigmoid formulation (one ACT table, zero table reloads):
    si=sig(a_i), sf=sig(a_f), sg=sig(2*a_g), so=sig(a_o)
    U = (sg-0.5)*si                  [= sig_i*tanh(a_g)/2]
    ch(t) = sf*ch(t-1) + U           [= c/2, via tensor_tensor_scan]
    tct' = sig(4*ch)                 [= (tanh(c)+1)/2]
    hdev = (tct'-0.5)*so             [= h/2; Wh pre-doubled, host doubles out]

Per-core layout: n-lane-major streams. PSUM banks A=[i|f], B=[g|o] (gate
chunk pairs as 128-wide matmul outputs, x-projection + h-projection
accumulated in PSUM). Sigmoid ACTs write f32 staging tiles [128, T]; U on
GpSimd (idle engine) with rebase-write to partitions 64:128 so that scan /
tct' / hmult all run at base 64 where sf / so already live. h_seq is a single
persistent bf16 buffer [128, 16*(T+1)] (even lanes rows 0:64, odd rows
64:128, col 0 = h(-1) = 0, writes shifted +1) - within-lane WAR ordering
makes one buffer race-free across sweeps.
"""

import os
import numpy as np
import ml_dtypes

BF16 = ml_dtypes.bfloat16

OBS = 32
HID = 64
T_FULL = 2048
B_FULL = 256
N_CORES = 8
BPC = B_FULL // N_CORES   # 32 batch lanes per core
NP = BPC // 2             # 16 lane pairs (even rows 0:64, odd rows 64:128)
KA = OBS + 1              # x rows incl ones-row
SWEEPS = int(os.environ.get("LSTM_SWEEPS", "5"))

_NC_CACHE = {}


# --------------------------------------------------------------------------
# Device program
# --------------------------------------------------------------------------
def build_nc(t_steps=T_FULL, sweeps=SWEEPS):
    import concourse.bass as bass
    import concourse.tile as tile
    import concourse.mybir as mybir
    from concourse.tile import add_dep_helper

    f32 = mybir.dt.float32
    bf16 = mybir.dt.bfloat16
    SIG = mybir.ActivationFunctionType.Sigmoid
    ADD = mybir.AluOpType.add
    MULT = mybir.AluOpType.mult

    T = t_steps
    TP1 = T + 1
    TQ = min(512, T)         # psum tile cols (one 2KB bank)
    nq = T // TQ
    BKC = min(512, TQ)       # bank-aligned matmul col group
    nbk = TQ // BKC

    nc = bass.Bass("TRN2", debug=False, num_devices=N_CORES,
                   enable_partition_id=False)

    # x: [KA, (n, t)] bf16 per core ([x; 1] rows, lane-major cols).
    x_dram = nc.dram_tensor("x", [KA, BPC * T], bf16, kind="ExternalInput")
    # Weights: cols 0:128 = WxA=[f|i], 128:256 = WxB=[o|g] (rows 0:KA,
    # g-cols doubled), 256:384 = WhA, 384:512 = WhB (all doubled, g-cols x4).
    wcat = nc.dram_tensor("wcat", [HID, 512], bf16, kind="ExternalInput")
    # Output: h_seq dump [64, BPC*(T+1)] bf16 (hdev = h/2, shifted +1).
    hs_dram = nc.dram_tensor("hs", [HID, BPC * TP1], bf16,
                             kind="ExternalOutput")

    with tile.TileContext(nc) as tc:
        from contextlib import ExitStack
        ctx = ExitStack()
        with ctx:
            wpool = ctx.enter_context(tc.tile_pool(name="weights", bufs=1))
            xpool = ctx.enter_context(tc.tile_pool(name="xstage", bufs=4))
            tApool = ctx.enter_context(tc.tile_pool(name="tA", bufs=2))
            tBpool = ctx.enter_context(tc.tile_pool(name="tB", bufs=2))
            Upool = ctx.enter_context(tc.tile_pool(name="U", bufs=1))
            chpool = ctx.enter_context(tc.tile_pool(name="ch", bufs=1))
            tcpool = ctx.enter_context(tc.tile_pool(name="tct", bufs=2))
            psA = ctx.enter_context(
                tc.tile_pool(name="psA", bufs=2, space="PSUM"))
            psB = ctx.enter_context(
                tc.tile_pool(name="psB", bufs=2, space="PSUM"))

            w_all = wpool.tile([HID, 512], bf16)
            nc.sync.dma_start(w_all[:, :], wcat[:, :])
            wxA = w_all[0:KA, 0:128]
            wxB = w_all[0:KA, 128:256]
            whA = w_all[0:HID, 256:384]
            whB = w_all[0:HID, 384:512]
            nc.tensor.ldweights(whA)

            # Persistent h/2 sequence, single buffer, all lanes at base 0.
            # memset once -> h^0 = 0; col 0 per lane stays h(-1) = 0 forever.
            h_seq = nc.alloc_sbuf_tensor("h_seq", [HID, BPC * TP1], bf16)
            # split: a single memset's element count must fit in 16 bits
            half = (BPC // 2) * TP1
            nc.vector.memset(h_seq[:][:, 0:half], 0.0)
            nc.vector.memset(h_seq[:][:, half:BPC * TP1], 0.0)

            for m in range(sweeps):
                for n in range(BPC):
                    hcol = n * TP1
                    tA = tApool.tile([128, T], f32, tag="tA")
                    tB = tBpool.tile([128, T], f32, tag="tB")
                    U = Upool.tile([HID, T], f32, tag="U")
                    for q in range(nq):
                        xs = xpool.tile([KA, TQ], bf16)
                        nc.sync.dma_start(
                            xs[:, :],
                            x_dram[:, n * T + q * TQ:n * T + (q + 1) * TQ])
                        bA = psA.tile([128, TQ], f32)
                        bB = psB.tile([128, TQ], f32)
                        for k in range(nbk):
                            cs = slice(k * BKC, (k + 1) * BKC)
                            rhs_h = h_seq[:][
                                0:HID,
                                hcol + q * TQ + k * BKC:
                                hcol + q * TQ + (k + 1) * BKC]
                            for bank, wx, wh in ((bA, wxA, whA),
                                                 (bB, wxB, whB)):
                                mmx = nc.tensor.matmul(
                                    bank[:, cs], lhsT=wx, rhs=xs[:, cs],
                                    start=True, stop=(m == 0),
                                    skip_group_check=True)
                                if m > 0:
                                    mmh = nc.tensor.matmul(
                                        bank[:, cs], lhsT=wh, rhs=rhs_h,
                                        start=False, stop=True,
                                        skip_group_check=True)
                                    add_dep_helper(
                                        mmh.ins, mmx.ins, sync=False,
                                        reason="accumulate after bank clear")
                        qs = slice(q * TQ, (q + 1) * TQ)
                        nc.scalar.activation(tA[:, qs], bA[:, :], SIG)
                        nc.scalar.activation(tB[:, qs], bB[:, :], SIG)
                        # U = (sg - 0.5) * si  (hi halves) -> rebase-write
                        # down to rows 0:64 where sf/so live
                        nc.vector.scalar_tensor_tensor(
                            U[:, qs], tB[64:128, qs], -0.5,
                            tA[64:128, qs], ADD, MULT)
                    # ch(t) = sf * ch(t-1) + U   (fp32 state)
                    ch = chpool.tile([HID, T], f32, tag="ch")
                    nc.vector.tensor_tensor_scan(
                        ch[:, :], tA[0:HID, :], U[:, :], 0.0,
                        MULT, ADD)
                    # tct' = sig(4*ch)
                    tct = tcpool.tile([HID, T], f32, tag="tct")
                    nc.scalar.activation(tct[:, :], ch[:, :], SIG, scale=4.0)
                    # hdev = (tct' - 0.5) * so -> h_seq cols shifted +1
                    nc.vector.scalar_tensor_tensor(
                        h_seq[:][0:HID, hcol + 1:hcol + 1 + T],
                        tct[:, :], -0.5, tB[0:HID, :], ADD, MULT)

            nc.sync.dma_start(hs_dram[:, :], h_seq[:][:, :])
    return nc


def _split_waits(nc, mybir, nmax=1):
    """This walrus accepts only one sync-wait per instruction: move excess
    waits onto preceding same-engine NOPs."""
    fn = nc.m.functions[0]
    for bb in fn.blocks:
        newlist = []
        for ins in bb.instructions:
            si = getattr(ins, "sync_info", None)
            if si is not None and si.on_wait and len(si.on_wait) > nmax:
                waits = list(si.on_wait)
                while len(waits) > nmax:
                    chunk, waits = waits[:nmax], waits[nmax:]
                    nop = mybir.InstNoOp(
                        name=nc.get_next_instruction_name(), ins=[], outs=[])
                    nop.engine = ins.engine
                    nop.sync_info = mybir.SyncInfo(on_wait=chunk, on_update=[])
                    newlist.append(nop)
                si.on_wait = waits
            newlist.append(ins)
        bb.instructions[:] = newlist


# --------------------------------------------------------------------------
# Host-side weight/input prep
# --------------------------------------------------------------------------
def _prep_weights(Wx, Wh, b):
    """Chunk pairs A=[i|f], B=[g|o]; g-cols doubled (sig(2a) form); bias as
    extra x-row; Wh doubled (rhs is h/2)."""
    H = HID
    idx = {"i": np.arange(0, H), "f": np.arange(H, 2 * H),
           "g": np.arange(2 * H, 3 * H), "o": np.arange(3 * H, 4 * H)}
    gscale = np.ones(4 * H, np.float32)
    gscale[idx["g"]] = 2.0
    Wxs = np.asarray(Wx, np.float32) * gscale
    Whs = np.asarray(Wh, np.float32) * gscale * 2.0
    bs = np.asarray(b, np.float32) * gscale
    Wxa = np.concatenate([Wxs, bs[None, :]], axis=0)  # [KA, 256]
    wcat = np.zeros((HID, 512), np.float32)
    wcat[0:KA, 0:64] = Wxa[:, idx["f"]]
    wcat[0:KA, 64:128] = Wxa[:, idx["i"]]
    wcat[0:KA, 128:192] = Wxa[:, idx["o"]]
    wcat[0:KA, 192:256] = Wxa[:, idx["g"]]
    wcat[:, 256:320] = Whs[:, idx["f"]]
    wcat[:, 320:384] = Whs[:, idx["i"]]
    wcat[:, 384:448] = Whs[:, idx["o"]]
    wcat[:, 448:512] = Whs[:, idx["g"]]
    return wcat.astype(BF16)


def _prep_x(y_core):
    """y_core [BPC, T, OBS] fp32 -> [KA, BPC*T] bf16, lane-major cols."""
    t_steps = y_core.shape[1]
    xa = np.empty((KA, BPC, t_steps), np.float32)
    xa[0:OBS] = y_core.transpose(2, 0, 1)
    xa[OBS] = 1.0
    return np.ascontiguousarray(
        xa.reshape(KA, BPC * t_steps).astype(BF16))


def kernel(y, Wx, Wh, b):
    from concourse.bass_utils import run_bass_kernel_spmd

    y = np.asarray(y)
    t_steps = y.shape[1]
    wcat = _prep_weights(Wx, Wh, b)

    key = t_steps
    if key not in _NC_CACHE:
        import concourse.mybir as mybir
        nc = build_nc(t_steps)
        _split_waits(nc, mybir)
        _NC_CACHE[key] = nc
    nc = _NC_CACHE[key]

    in_maps = []
    for c in range(N_CORES):
        in_maps.append({"wcat": wcat,
                        "x": _prep_x(y[c * BPC:(c + 1) * BPC])})

    res = run_bass_kernel_spmd(
        nc, in_maps, core_ids=list(range(N_CORES)),
        trace=bool(int(os.environ.get("LSTM_TRACE", "0"))))

    out = np.empty((B_FULL, t_steps, HID), np.float32)
    for c in range(N_CORES):
        hs = res.results[c]["hs"].astype(np.float32)  # [64, BPC*(T+1)]
        hs = hs.reshape(HID, BPC, t_steps + 1)
        out[c * BPC:(c + 1) * BPC] = hs[:, :, 1:].transpose(1, 2, 0) * 2.0
    globals()["_LAST_RESULT"] = res
    return out
